# revision 4
# baseline (speedup 1.0000x reference)
"""Trainium2 Bass kernel for BlockAttnResLayer — bf16 V/FFN, fp32 h accum.

Data-parallel over tokens (512/core).  Host-side casts: V (blocks+partial) and
both FFN weights to bf16, proj_w*norm_scale premultiplied to bf16; a separate
fp32 copy of the partial block feeds the residual.  h_out is produced bf16 and
upcast to fp32 on the host.

Per-core work:
  Phase A (attention): per 128-token tile: 9 bf16 V loads (SP queue), squares
  on ACT (accum -> ss9), dots on DVE bf16-2x (accum -> dp9), softmax smalls,
  weighted sum -> h_t bf16 (DVE 2x), PE transposes -> hT bf16.
  MM1-A (f-chunks over token half A, N=256) interleaves with tiles 2,3.
  Phase B: MM1-B + MM2 (64-chunk PSUM chains per (token-tile, quarter)) +
  fp32 residual -> np_out.

Queues: V on sync(SP); w1 + partial + np_out on gpsimd(Pool); h_out on ACT.
"""
import numpy as np
from contextlib import ExitStack

import ml_dtypes

import concourse.bass as bass
import concourse.bacc as bacc
import concourse.tile as tile
from concourse import mybir
from concourse.bass_utils import run_bass_kernel_spmd
from concourse.masks import make_identity

f32 = mybir.dt.float32
bf16 = mybir.dt.bfloat16
AF = mybir.ActivationFunctionType
ALU = mybir.AluOpType

N_CORES = 8
NB = 8            # completed blocks
N1 = 9            # blocks + partial
B, T, D, F = 2, 2048, 2048, 8192
TOK = B * T       # 4096
TPC = TOK // N_CORES  # 512 tokens per core
P = 128
TT = TPC // P     # 4 token tiles per core
TH = TPC // 2     # 256-token halves
DC = D // P       # 16 d-chunks
FC = F // P       # 64 f-chunks
NQ = D // 512     # 4 output column quarters
EPS = 1e-8


def retile_w1(w1: np.ndarray) -> np.ndarray:
    """[D, F] -> [FC, P, DC, P] with w1t[fc, p, kc, q] = W1[kc*P+p, fc*P+q]."""
    return np.ascontiguousarray(
        w1.reshape(DC, P, FC, P).transpose(2, 1, 0, 3))


def build_nc(n_reps: int = 1, gelu: bool = True, overlap: bool = True):
    act_fn = AF.Gelu_apprx_tanh if gelu else AF.Copy
    nc = bacc.Bacc("TRN2", target_bir_lowering=False, debug=False, num_devices=N_CORES)
    vb = nc.dram_tensor("vb", [N1, TPC, D], bf16, kind="ExternalInput").ap()
    pb32 = nc.dram_tensor("pb32", [TPC, D], f32, kind="ExternalInput").ap()
    w1 = nc.dram_tensor("w1", [FC, P, DC, P], bf16, kind="ExternalInput").ap()
    w2 = nc.dram_tensor("w2", [F, D], bf16, kind="ExternalInput").ap()
    pw = nc.dram_tensor("pw", [D], bf16, kind="ExternalInput").ap()
    h_out = nc.dram_tensor("h_out", [TPC, D], f32, kind="ExternalOutput").ap()
    np_out = nc.dram_tensor("np_out", [TPC, D], f32, kind="ExternalOutput").ap()

    h_out_t = h_out.rearrange("(tt p) d -> tt p d", p=P)

    with tile.TileContext(nc) as tc, ExitStack() as ctx:
        outer = ctx.enter_context(tc.tile_pool(name="outer", bufs=1))
        pw_b = outer.tile([P, D], bf16)
        # transposed h in two token-halves: hTs[half][k] is [128 d, 256 t] bf16
        hTs = [[outer.tile([P, TH], bf16, name=f"hT{hf}_{k}") for k in range(DC)]
               for hf in range(2)]
        actpA = ctx.enter_context(tc.tile_pool(name="actpA", bufs=FC))
        w1p = ctx.enter_context(tc.tile_pool(name="w1p", bufs=10))
        ps1p = ctx.enter_context(tc.tile_pool(name="ps1p", bufs=2, space="PSUM"))

        def mm1_half(hf, fc, dst, w1t=None):
            """One f-chunk of MM1 over token half hf -> act -> dst (SBUF bf16)."""
            if w1t is None:
                w1t = w1p.tile([P, DC, P], bf16, name="w1t")
                nc.gpsimd.dma_start(out=w1t, in_=w1[fc])
            ps1 = ps1p.tile([P, TH], f32, name="ps1")
            for k in range(DC):
                nc.tensor.matmul(ps1[:], lhsT=w1t[:, k, :], rhs=hTs[hf][k][:],
                                 start=(k == 0), stop=(k == DC - 1))
            nc.scalar.activation(dst[:], ps1[:], act_fn)
            return w1t

        for _rep in range(n_reps):
            acts_a = [actpA.tile([P, TH], bf16, name="aa") for _ in range(FC)]
            # ---------------- Phase A: block attention -> h, hT ----------------
            with ExitStack() as ctxA:
                vpool = ctxA.enter_context(tc.tile_pool(name="vpool", bufs=12))
                spool = ctxA.enter_context(tc.tile_pool(name="spool", bufs=1))
                sqsb = ctxA.enter_context(tc.tile_pool(name="sqsb", bufs=1))
                small = ctxA.enter_context(tc.tile_pool(name="small", bufs=4))
                hpool = ctxA.enter_context(tc.tile_pool(name="hpool", bufs=2))
                psumT = ctxA.enter_context(tc.tile_pool(name="psumT", bufs=1, space="PSUM"))
                consts = ctxA.enter_context(tc.tile_pool(name="consts", bufs=1))
                pswp = ctxA.enter_context(tc.tile_pool(name="pswp", bufs=4, space="PSUM"))
                diagp = ctxA.enter_context(tc.tile_pool(name="diagp", bufs=N1))

                ident = consts.tile([P, P], f32)
                make_identity(nc, ident)
                identb = consts.tile([P, P], bf16)
                make_identity(nc, identb)
                eps_t = consts.tile([P, 1], f32)
                nc.vector.memset(eps_t, EPS)
                pw_bc = bass.AP(tensor=pw.tensor, offset=pw.offset,
                                ap=[[0, P], *pw.ap])
                nc.gpsimd.dma_start(out=pw_b, in_=pw_bc)

                state = {}

                def attn_pre(tt, dve_sq=False):
                    """V loads + squares (ACT or DVE) + dots (DVE)."""
                    ss9 = small.tile([P, N1], f32, name="ss9")
                    dp9 = small.tile([P, N1], f32, name="dp9")
                    vts = []
                    for n in range(N1):
                        v = vpool.tile([P, D], bf16, name="vt")
                        nc.sync.dma_start(out=v, in_=vb[n, tt * P:(tt + 1) * P, :])
                        vts.append(v)
                        if dve_sq:
                            dsq = spool.tile([P, D], bf16, name="dsc")
                            nc.vector.scalar_tensor_tensor(
                                out=dsq[:], in0=v[:], scalar=1.0, in1=v[:],
                                op0=ALU.mult, op1=ALU.mult,
                                accum_out=ss9[:, n:n + 1])
                        else:
                            sq = sqsb.tile([P, D], bf16, name="sq")
                            nc.scalar.activation(sq[:], v[:], AF.Square,
                                                 accum_out=ss9[:, n:n + 1])
                        dsc = spool.tile([P, D], bf16, name="dsc")
                        nc.vector.scalar_tensor_tensor(
                            out=dsc[:], in0=v[:], scalar=1.0, in1=pw_b[:],
                            op0=ALU.mult, op1=ALU.mult, accum_out=dp9[:, n:n + 1])
                    state[tt] = (ss9, dp9, vts)

                def attn_sm(tt, pe_wsum=False):
                    """Softmax smalls + weighted sum (DVE, or PE via diag)."""
                    ss9, dp9, vts = state[tt]
                    rms9 = small.tile([P, N1], f32, name="rms9")
                    nc.scalar.activation(rms9[:], ss9[:], AF.Sqrt,
                                         bias=eps_t[:], scale=1.0 / D)
                    inv9 = small.tile([P, N1], f32, name="inv9")
                    nc.vector.reciprocal(inv9[:], rms9[:])
                    lg9 = small.tile([P, N1], f32, name="lg9")
                    nc.vector.tensor_mul(lg9[:], dp9[:], inv9[:])
                    mx1 = small.tile([P, 1], f32, name="mx1")
                    nc.vector.tensor_reduce(mx1[:], lg9[:], axis=mybir.AxisListType.X,
                                            op=ALU.max)
                    nc.vector.tensor_scalar_sub(lg9[:], lg9[:], mx1[:])
                    e9 = small.tile([P, N1], f32, name="e9")
                    se1 = small.tile([P, 1], f32, name="se1")
                    nc.scalar.activation(e9[:], lg9[:], AF.Exp, accum_out=se1[:])
                    invs = small.tile([P, 1], f32, name="invs")
                    nc.vector.reciprocal(invs[:], se1[:])
                    al9 = small.tile([P, N1], f32, name="al9")
                    nc.vector.tensor_scalar_mul(al9[:], e9[:], invs[:])

                    if pe_wsum:
                        # weighted sum on PE: psw[db] += diag(alpha_n) @ V_n[:, db]
                        # n-outer so each V tile is fully consumed (and its
                        # vpool slot freed) after its 4 matmuls.
                        psws = [pswp.tile([P, 512], f32, name="psw")
                                for _ in range(4)]
                        for n in range(N1):
                            dg = diagp.tile([P, P], bf16, name="dg")
                            nc.scalar.activation(dg[:], identb[:], AF.Copy,
                                                 scale=al9[:, n:n + 1])
                            for db in range(4):
                                nc.tensor.matmul(
                                    psws[db][:], lhsT=dg[:],
                                    rhs=vts[n][:, db * 512:(db + 1) * 512],
                                    start=(n == 0), stop=(n == N1 - 1))
                        state[tt] = psws
                    else:
                        h_t = hpool.tile([P, D], f32, name="ht")
                        nc.vector.tensor_scalar_mul(h_t[:], vts[0][:], al9[:, 0:1])
                        for n in range(1, N1):
                            nc.vector.scalar_tensor_tensor(
                                out=h_t[:], in0=vts[n][:], scalar=al9[:, n:n + 1],
                                in1=h_t[:], op0=ALU.mult, op1=ALU.add)
                        state[tt] = h_t

                def attn_tp(tt):
                    """PE transposes + hT copies; h_out store last (ACT queue)."""
                    h_t = state.pop(tt)
                    if isinstance(h_t, list):
                        psws, h_t = h_t, hpool.tile([P, D], f32, name="ht")
                        for db in range(4):
                            nc.scalar.activation(
                                h_t[:, db * 512:(db + 1) * 512], psws[db][:],
                                AF.Copy)
                    hf, col = divmod(tt, 2)
                    for k in range(DC):
                        pst = psumT.tile([P, P], f32, name="pst")
                        nc.tensor.transpose(pst[:], h_t[:, k * P:(k + 1) * P], ident[:])
                        nc.scalar.activation(
                            hTs[hf][k][:, col * P:(col + 1) * P], pst[:], AF.Copy)
                    nc.scalar.dma_start(out=h_out_t[tt], in_=h_t[:])

                def mm1a(lo, hi):
                    if overlap:
                        for fc in range(lo, hi):
                            mm1_half(0, fc, acts_a[fc])

                attn_pre(0)
                attn_sm(0, pe_wsum=True)
                attn_pre(1)
                attn_tp(0)
                attn_sm(1, pe_wsum=True)
                attn_tp(1)
                attn_pre(2, dve_sq=True)
                mm1a(0, 24)
                attn_sm(2, pe_wsum=True)
                mm1a(24, 32)
                attn_tp(2)
                attn_pre(3, dve_sq=True)
                mm1a(32, 56)
                attn_sm(3, pe_wsum=True)
                mm1a(56, FC)
                attn_tp(3)

            # ---------------- Phase B: FFN (bf16) + residual ----------------
            with ExitStack() as ctxB:
                actpB = ctxB.enter_context(tc.tile_pool(name="actpB", bufs=FC))
                w2p = ctxB.enter_context(tc.tile_pool(name="w2p", bufs=4))
                evp = ctxB.enter_context(tc.tile_pool(name="evp", bufs=4))
                ptp = ctxB.enter_context(tc.tile_pool(name="ptp", bufs=8))
                ps2p = ctxB.enter_context(tc.tile_pool(name="ps2p", bufs=6, space="PSUM"))

                acts_b = [actpB.tile([P, TH], bf16, name="ab") for _ in range(FC)]
                acts = [acts_a, acts_b]
                for fc in range(FC):
                    if not overlap:
                        w1t = mm1_half(0, fc, acts[0][fc])
                        mm1_half(1, fc, acts[1][fc], w1t=w1t)
                    else:
                        mm1_half(1, fc, acts[1][fc])

                # MM2 + residual: np = partial + act.T @ W2
                for q in range(NQ):
                    pts = []
                    for m in range(TT):
                        pt = ptp.tile([P, 512], f32, name="pt")
                        nc.gpsimd.dma_start(
                            out=pt,
                            in_=pb32[m * P:(m + 1) * P, q * 512:(q + 1) * 512])
                        pts.append(pt)
                    ps2 = [ps2p.tile([P, 512], f32, name="ps2") for _ in range(TT)]
                    for fc in range(FC):
                        w2t = w2p.tile([P, 512], bf16, name="w2t")
                        nc.sync.dma_start(
                            out=w2t,
                            in_=w2[fc * P:(fc + 1) * P, q * 512:(q + 1) * 512])
                        for m in range(TT):
                            hf, col = divmod(m, 2)
                            nc.tensor.matmul(
                                ps2[m][:],
                                lhsT=acts[hf][fc][:, col * P:(col + 1) * P],
                                rhs=w2t[:],
                                start=(fc == 0), stop=(fc == FC - 1))
                    for m in range(TT):
                        ev = evp.tile([P, 512], f32, name="ev")
                        nc.vector.tensor_add(ev[:], ps2[m][:], pts[m][:])
                        nc.gpsimd.dma_start(
                            out=np_out[m * P:(m + 1) * P, q * 512:(q + 1) * 512],
                            in_=ev[:])

    nc.compile()
    return nc


def make_in_maps(inputs):
    blocks = np.ascontiguousarray(
        np.asarray(inputs["blocks"], dtype=np.float32)).reshape(NB, TOK, D)
    pb = np.ascontiguousarray(
        np.asarray(inputs["partial_block"], dtype=np.float32)).reshape(TOK, D)
    w1r = retile_w1(np.asarray(inputs["ffn_w1"], dtype=np.float32).astype(ml_dtypes.bfloat16))
    w2r = np.asarray(inputs["ffn_w2"], dtype=np.float32).astype(ml_dtypes.bfloat16)
    pwf = (np.asarray(inputs["proj_w"], dtype=np.float32)
           * np.asarray(inputs["norm_scale"], dtype=np.float32))
    pwr = pwf.astype(ml_dtypes.bfloat16)

    in_maps = []
    for c in range(N_CORES):
        sl = slice(c * TPC, (c + 1) * TPC)
        vbc = np.concatenate([blocks[:, sl], pb[None, sl]],
                             axis=0).astype(ml_dtypes.bfloat16)
        in_maps.append({"vb": vbc, "pb32": np.ascontiguousarray(pb[sl]),
                        "w1": w1r, "w2": w2r, "pw": pwr})
    return in_maps


_NC = None


def _get_nc():
    global _NC
    if _NC is None:
        _NC = build_nc()
    return _NC


def kernel(blocks, partial_block, proj_w, norm_scale, ffn_w1, ffn_w2):
    in_maps = make_in_maps(dict(blocks=blocks, partial_block=partial_block,
                                proj_w=proj_w, norm_scale=norm_scale,
                                ffn_w1=ffn_w1, ffn_w2=ffn_w2))
    nc = _get_nc()
    res = run_bass_kernel_spmd(nc, in_maps, list(range(N_CORES)))
    h = np.concatenate([r["h_out"] for r in res.results],
                       axis=0).reshape(B, T, D)
    npar = np.concatenate([r["np_out"] for r in res.results], axis=0).reshape(B, T, D)
    return h, npar
